# revision 14
# baseline (speedup 1.0000x reference)
"""Trainium2 Bass kernel for nn_DMGAGRUcell (GRU cell with graph-conv gates).

Math (per batch b):
  x    = [inputs | hx]                      (N, 66)
  ru   = sigmoid(x W0 + (S x) W1 + (adp x) W2);  r, u = split(ru)
  x'   = [inputs | r*hx]
  c    = tanh(x' Wc0 + (S x') Wc1 + (adp x') Wc2)
  out  = u*hx + (1-u)*c

Sharding: 2 batches per core x 8 cores (data parallel over B=16).

Device strategy:
  - All four N x N streaming products run as fp8e4 DoubleRow matmuls
    (0.5 cycles/row, 2 k-chunks per instruction).  S is scaled x256 and
    adp x32768 so fp8 values stay out of the subnormal range; the scales
    are folded into the gate weights on the host.
  - gconv1 is classic: streams produce x1T/x2T (bf16, feature-major),
    small matmuls per 512-slab accumulate the ru pre-activation.  For
    the late batch the x0/x1 terms are pre-accumulated into SBUF early
    (rupT) and re-injected with an identity matmul, so only two matmuls
    per slab remain after its adp pass lands.
  - gconv2 is weight-folded: y1 = x'(Wc1*LAM/256), y2 = x'(Wc2*LAM/32768)
    are computed node-major (tiny matmuls, inp-part + rh-part) and
    quantized to fp8; the S/adp streams then accumulate LAM*c_pre
    directly in PSUM (with the Wc0*LAM direct term), and tanh applies
    scale 1/LAM.  No second-gconv drains, no transposes.
  - hx lives at partitions 0:64 and the 2 input rows at 64:66 of one
    packed tile (HW requires 32-aligned partition bases); weight rows
    are permuted on the host to match, so every matmul/elementwise op
    has legal base pairs.  PSUM: two 4-slot rings whose allocation order
    matches the pass windows.
"""

import numpy as np
import ml_dtypes

BF16 = ml_dtypes.bfloat16
FP8 = ml_dtypes.float8_e4m3fn

N = 2048
B = 16
D_IN = 2
UNITS = 64
F = 66
B_LOC = 2          # batches per core
N_CORES = 8
KC = 16            # k chunks of 128 nodes
KP = 8             # k-chunk pairs (DoubleRow)
NS = 4             # 512-wide output slabs
FPAD = 80          # padded feature count (DoubleRow needs step % 16 == 0)

S_SCALE = 256.0    # fp8 scale for the sparse support matrix
A_SCALE = 32768.0  # fp8 scale for the adaptive adjacency (2048 * 16)
LAM = 32768.0      # common fixed-point scale of the gconv2 PSUM accumulation

_CACHE = {}


def _build():
    if "nc" in _CACHE:
        return _CACHE["nc"]

    from contextlib import ExitStack
    import concourse.mybir as mybir
    import concourse.tile as tile
    from concourse import bacc

    f32 = mybir.dt.float32
    bf = mybir.dt.bfloat16
    f8 = mybir.dt.float8e4
    AF = mybir.ActivationFunctionType
    DR = mybir.MatmulPerfMode.DoubleRow

    nc = bacc.Bacc("TRN2", target_bir_lowering=False, debug=False,
                   num_devices=N_CORES)

    adp_d = nc.dram_tensor("adpT", [B_LOC, KP, 128, 2, N], f8, kind="ExternalInput")
    s_d = nc.dram_tensor("sT", [KP, 128, 2, N], f8, kind="ExternalInput")
    xnm_d = nc.dram_tensor("xnm", [128, B_LOC, KC, FPAD], f8, kind="ExternalInput")
    # hxi: rows 0:64 = hx (feature-major), rows 64:66 = inputs
    hxi_d = nc.dram_tensor("hxi", [B_LOC, F, N], bf, kind="ExternalInput")
    # wblob cols: [0:384] wru (3x128, rows hxi-permuted for m=0),
    # [384:512] wcy (2x64), [512:576] wc0, [576:704] identity(128).
    # wcy/wc0 rows: 0:64 hx-part, 64:66 inp-part (hxi layout).
    wb_d = nc.dram_tensor("wblob", [128, 704], bf, kind="ExternalInput")
    out_d = nc.dram_tensor("outT", [B_LOC, UNITS, N], bf, kind="ExternalOutput")

    with tile.TileContext(nc) as tc, ExitStack() as ctx:
        cpool = ctx.enter_context(tc.tile_pool(name="cpool", bufs=1))
        spool = ctx.enter_context(tc.tile_pool(name="spool", bufs=1))
        apool = ctx.enter_context(tc.tile_pool(name="apool", bufs=1))
        wk = ctx.enter_context(tc.tile_pool(name="wk", bufs=1))
        pp = ctx.enter_context(tc.tile_pool(name="pp", bufs=4, space="PSUM"))

        def sl(s):
            return slice(s * 512, (s + 1) * 512)

        # ---- tiles (SBUF residency) ----
        xnm = wk.tile([128, B_LOC, KC, FPAD], f8, tag="xnm", name="xnm")
        hxi = [wk.tile([F, N], bf, tag=f"hxi{b}", name=f"hxi{b}")
               for b in range(B_LOC)]
        s2 = [spool.tile([128, 2, N], f8, tag=f"s{kp}", name=f"s{kp}")
              for kp in range(KP)]
        a2 = [[apool.tile([128, 2, N], f8, tag=f"a{b}_{kp}", name=f"a{b}_{kp}")
               for kp in range(KP)] for b in range(B_LOC)]
        wb = cpool.tile([128, 704], bf, tag="wb", name="wb")

        def wru_hx0():
            return wb[0:UNITS, 0:128]

        def wru_inp0():
            return wb[UNITS:F, 0:128]

        def wru(m):
            return wb[0:F, 128 * m:128 * (m + 1)]

        def wcyR(j):
            return wb[0:UNITS, 384 + 64 * j:384 + 64 * (j + 1)]

        def wcyI(j):
            return wb[UNITS:F, 384 + 64 * j:384 + 64 * (j + 1)]

        wc0R = wb[0:UNITS, 512:576]
        wc0I = wb[UNITS:F, 512:576]
        ident = wb[:, 576:704]

        x1T = [wk.tile([F, N], bf, tag=f"x1T{b}", name=f"x1T{b}") for b in range(B_LOC)]
        x2T = [wk.tile([F, N], bf, tag=f"x2T{b}", name=f"x2T{b}") for b in range(B_LOC)]
        rupT = wk.tile([128, N], bf, tag="rupT", name="rupT")
        rhT = [wk.tile([UNITS, N], bf, tag=f"rhT{b}", name=f"rhT{b}") for b in range(B_LOC)]
        ract = [wk.tile([UNITS, N], bf, tag=f"ract{b}", name=f"ract{b}") for b in range(B_LOC)]
        uact = [wk.tile([UNITS, N], bf, tag=f"uact{b}", name=f"uact{b}") for b in range(B_LOC)]
        cT = [wk.tile([UNITS, N], bf, tag=f"cT{b}", name=f"cT{b}") for b in range(B_LOC)]
        outT = [wk.tile([UNITS, N], bf, tag=f"outT{b}", name=f"outT{b}") for b in range(B_LOC)]
        y1nm = [wk.tile([128, KC, UNITS], f8, tag=f"y1nm{b}", name=f"y1nm{b}")
                for b in range(B_LOC)]
        y2nm = [wk.tile([128, KC, UNITS], f8, tag=f"y2nm{b}", name=f"y2nm{b}")
                for b in range(B_LOC)]

        # ---- DMA issue order == HBM arrival order (one serialized bus,
        # ~0.65us HWDGE serialization per DMA, so smalls are packed).
        nc.sync.dma_start(wb[:], wb_d[:])
        nc.sync.dma_start(xnm[:], xnm_d[:])
        for kp in range(KP):
            nc.sync.dma_start(s2[kp][:], s_d[kp])
        nc.sync.dma_start(hxi[0][:], hxi_d[0])
        nc.sync.dma_start(hxi[1][:], hxi_d[1])
        for kp in range(KP):
            nc.sync.dma_start(a2[0][kp][:], adp_d[0, kp])
        for kp in range(KP):
            nc.sync.dma_start(a2[1][kp][:], adp_d[1, kp])

        # Warm the ACT sigmoid/tanh function table off the critical path
        # (the first Sigmoid otherwise pays a ~1.3us table load mid-kernel).
        dum = cpool.tile([1, 2], f32, tag="dum", name="dum")
        nc.scalar.activation(dum[0:1, 0:1], wb[0:1, 0:1], AF.Sigmoid)

        def drain(dst, src, s):
            # PSUM -> SBUF copies: only DVE and ACT may read PSUM.
            if s % 2 == 0:
                nc.vector.tensor_copy(dst, src)
            else:
                nc.scalar.activation(dst, src, AF.Copy)

        def dr_mm(ps, lhsT_tile, rhs_tile, s, start, stop):
            nc.tensor.matmul(ps[:], lhsT_tile, rhs_tile[:, :, sl(s)],
                             start=start, stop=stop, perf_mode=DR)

        def xnm_pair(b, kp):
            return xnm[:, b, 2 * kp:2 * kp + 2, :]

        def ru_open_mms(ps, b, s, stop_last):
            # x0 term (split into hx + inp parts) and the x1 term.
            nc.tensor.matmul(ps[:], wru_hx0(), hxi[b][0:UNITS, sl(s)],
                             start=True, stop=False)
            nc.tensor.matmul(ps[:], wru_inp0(), hxi[b][UNITS:F, sl(s)],
                             start=False, stop=False)
            nc.tensor.matmul(ps[:], wru(1), x1T[b][:, sl(s)],
                             start=False, stop=stop_last)

        def ymm(b, psY, chunks, which):
            # y{1,2} = x' @ Wcy_{1,2}, node-major: per 128-node chunk,
            # two tiny matmuls (inp rows + rh rows) accumulate [128, 64].
            for k in chunks:
                ck = slice(128 * k, 128 * (k + 1))
                dst = psY[k // 8][:, k % 8, :]
                nc.tensor.matmul(dst, hxi[b][UNITS:F, ck], wcyI(which),
                                 start=True, stop=False)
                nc.tensor.matmul(dst, rhT[b][:, ck], wcyR(which),
                                 start=False, stop=True)

        def ynm_copies(b, psY1, psY2, y2_on_act=False):
            # quantize y1/y2 to fp8 node-major, 4-chunk granularity; all of
            # y1 first (the S stream consumes it first).
            for yd, ps, act in ((y1nm[b], psY1, False), (y2nm[b], psY2, y2_on_act)):
                for q in range(4):
                    src = ps[q // 2][:, 4 * (q % 2):4 * (q % 2) + 4, :]
                    dst = yd[:, 4 * q:4 * q + 4, :]
                    if act:
                        nc.scalar.activation(dst, src, AF.Copy)
                    else:
                        nc.vector.tensor_copy(dst, src)

        def cterm0(psC, b, s):
            nc.tensor.matmul(psC[s][:], wc0I, hxi[b][UNITS:F, sl(s)],
                             start=True, stop=False)
            nc.tensor.matmul(psC[s][:], wc0R, rhT[b][:, sl(s)],
                             start=False, stop=False)

        def final_ops(b, s):
            nc.vector.tensor_sub(outT[b][:, sl(s)], hxi[b][0:UNITS, sl(s)],
                                 cT[b][:, sl(s)])
            nc.vector.tensor_mul(outT[b][:, sl(s)], uact[b][:, sl(s)],
                                 outT[b][:, sl(s)])
            nc.vector.tensor_add(outT[b][:, sl(s)], outT[b][:, sl(s)],
                                 cT[b][:, sl(s)])

        # ============ gconv1 S passes, both batches pair-paced ============
        psS0 = [pp.tile([FPAD, 512], f32, tag="pA", name=f"psS0_{s}")
                for s in range(NS)]
        psS1 = [pp.tile([FPAD, 512], f32, tag="pB", name=f"psS1_{s}")
                for s in range(NS)]
        for kp in range(KP):
            for s in range(NS):
                dr_mm(psS0[s], xnm_pair(0, kp), s2[kp], s, kp == 0, kp == KP - 1)
            for s in range(NS):
                dr_mm(psS1[s], xnm_pair(1, kp), s2[kp], s, kp == 0, kp == KP - 1)
        for s in range(NS):
            drain(x1T[0][:, sl(s)], psS0[s][0:F, :], s)
        for s in range(NS):
            drain(x1T[1][:, sl(s)], psS1[s][0:F, :], s + 1)

        # b1 ru partial: x0 + x1 terms accumulated early, drained to SBUF.
        rup1 = [pp.tile([128, 512], f32, tag="pB", name=f"rup1_{s}")
                for s in range(NS)]
        for s in range(NS):
            ru_open_mms(rup1[s], 1, s, stop_last=True)
        for s in range(NS):
            drain(rupT[:, sl(s)], rup1[s][:], s)

        # ru b0 opens with the x0/x1 terms (ring B, freed by sigmoids).
        ru0 = [pp.tile([128, 512], f32, tag="pB", name=f"ru0_{s}")
               for s in range(NS)]
        for s in range(NS):
            ru_open_mms(ru0[s], 0, s, stop_last=False)

        # ============ gconv1 adp pass b0 (pair-paced, ring A) ============
        psA0 = [pp.tile([FPAD, 512], f32, tag="pA", name=f"psA0_{s}")
                for s in range(NS)]
        for kp in range(KP):
            for s in range(NS):
                dr_mm(psA0[s], xnm_pair(0, kp), a2[0][kp], s, kp == 0, kp == KP - 1)

        # Per-slab: drain x2T -> close ru -> sigmoid(r) -> rh, pipelined.
        for s in range(NS):
            drain(x2T[0][:, sl(s)], psA0[s][0:F, :], s)
            nc.tensor.matmul(ru0[s][:], wru(2), x2T[0][:, sl(s)],
                             start=False, stop=True)
            nc.scalar.activation(ract[0][:, sl(s)], ru0[s][0:UNITS, :],
                                 AF.Sigmoid)
            nc.vector.tensor_mul(rhT[0][:, sl(s)], ract[0][:, sl(s)],
                                 hxi[0][0:UNITS, sl(s)])
        # u sigmoids as a block (off the rh critical path; they release the
        # ru0 slots that psC0 reuses).
        for s in range(NS):
            nc.scalar.activation(uact[0][:, sl(s)], ru0[s][UNITS:128, :],
                                 AF.Sigmoid)

        # y1/y2 for b0 (ring A after psA0's drains release its slots).
        psY0 = [pp.tile([128, 8, UNITS], f32, tag="pA", name=f"psY0_{i}")
                for i in range(4)]
        ymm(0, psY0[0:2], range(KC), 0)
        ymm(0, psY0[2:4], range(KC), 1)
        ynm_copies(0, psY0[0:2], psY0[2:4])

        # b1 gconv1 adp pass (ring A after psY0's ynm copies release slots;
        # the early pairs are resident by then, the rest are DMA-paced).
        psA1 = [pp.tile([FPAD, 512], f32, tag="pA", name=f"psA1_{s}")
                for s in range(NS)]

        def a1_block(kp):
            for s in range(NS):
                dr_mm(psA1[s], xnm_pair(1, kp), a2[1][kp], s,
                      kp == 0, kp == KP - 1)

        a1_block(0)
        a1_block(1)

        # ====== b0 gconv2 (ring B): kp-major S-section consumes ynm as it
        # lands; slab-major adp section pipelines tanh/finals; b1's adp
        # pairs and ru close are woven in by DMA arrival time. ======
        psC0 = [pp.tile([UNITS, 512], f32, tag="pB", name=f"psC0_{s}")
                for s in range(NS)]
        for s in range(NS):
            cterm0(psC0, 0, s)
        for kp in range(KP):
            for s in range(NS):
                dr_mm(psC0[s], y1nm[0][:, 2 * kp:2 * kp + 2, :], s2[kp], s,
                      False, False)
            if kp == 1:
                a1_block(2)
            if kp == 4:
                a1_block(3)
            if kp == 7:
                a1_block(4)
        ru1 = [pp.tile([128, 512], f32, tag="pA", name=f"ru1_{q}")
               for q in range(NS)]
        for s in range(NS):
            for kp in range(KP):
                dr_mm(psC0[s], y2nm[0][:, 2 * kp:2 * kp + 2, :], a2[0][kp], s,
                      False, kp == KP - 1)
            nc.scalar.activation(cT[0][:, sl(s)], psC0[s][:],
                                 AF.Tanh, scale=1.0 / LAM)
            nc.gpsimd.tensor_sub(outT[0][:, sl(s)], hxi[0][0:UNITS, sl(s)],
                                 cT[0][:, sl(s)])
            nc.gpsimd.tensor_mul(outT[0][:, sl(s)], uact[0][:, sl(s)],
                                 outT[0][:, sl(s)])
            nc.gpsimd.tensor_add(outT[0][:, sl(s)], outT[0][:, sl(s)],
                                 cT[0][:, sl(s)])
            if s == 0:
                a1_block(5)
            if s == 1:
                a1_block(6)
                nc.sync.dma_start(out_d[0, :, 0:1024], outT[0][:, 0:1024])
            if s == 2:
                a1_block(7)
                # b1 gconv1 close: drains all-DVE, ident re-injects the
                # early partial, then the x2 term; r/u sigmoids follow.
                for q in range(NS):
                    drain(x2T[1][:, sl(q)], psA1[q][0:F, :], q)
                    nc.tensor.matmul(ru1[q][:], ident, rupT[:, sl(q)],
                                     start=True, stop=False)
                    nc.tensor.matmul(ru1[q][:], wru(2), x2T[1][:, sl(q)],
                                     start=False, stop=True)
                    nc.scalar.activation(ract[1][:, sl(q)], ru1[q][0:UNITS, :],
                                         AF.Sigmoid)
                    nc.vector.tensor_mul(rhT[1][:, sl(q)], ract[1][:, sl(q)],
                                         hxi[1][0:UNITS, sl(q)])
                    nc.scalar.activation(uact[1][:, sl(q)],
                                         ru1[q][UNITS:128, :], AF.Sigmoid)
            if s == 3:
                nc.sync.dma_start(out_d[0, :, 1024:2048], outT[0][:, 1024:2048])

        # ---- b1 y/ynm (psY1 on ring A after ru1's sigmoids) ----
        psY1 = [pp.tile([128, 8, UNITS], f32, tag="pA", name=f"psY1_{i}")
                for i in range(4)]
        ymm(1, psY1[0:2], range(KC), 0)
        ymm(1, psY1[2:4], range(KC), 1)
        for q in range(4):
            drain(y1nm[1][:, 4 * q:4 * q + 4, :],
                  psY1[q // 2][:, 4 * (q % 2):4 * (q % 2) + 4, :], q)
        for q in range(4):
            drain(y2nm[1][:, 4 * q:4 * q + 4, :],
                  psY1[2 + q // 2][:, 4 * (q % 2):4 * (q % 2) + 4, :], q + 1)

        # ---- b1 gconv2 (ring B): kp-major S, slab-major adp + tail ----
        psC1 = [pp.tile([UNITS, 512], f32, tag="pB", name=f"psC1_{s}")
                for s in range(NS)]
        for s in range(NS):
            cterm0(psC1, 1, s)
        for kp in range(KP):
            for s in range(NS):
                dr_mm(psC1[s], y1nm[1][:, 2 * kp:2 * kp + 2, :], s2[kp], s,
                      False, False)
        for s in range(NS):
            for kp in range(KP):
                dr_mm(psC1[s], y2nm[1][:, 2 * kp:2 * kp + 2, :], a2[1][kp], s,
                      False, kp == KP - 1)
            nc.scalar.activation(cT[1][:, sl(s)], psC1[s][:],
                                 AF.Tanh, scale=1.0 / LAM)
            nc.vector.tensor_sub(outT[1][:, sl(s)], hxi[1][0:UNITS, sl(s)],
                                 cT[1][:, sl(s)])
            nc.vector.tensor_mul(outT[1][:, sl(s)], uact[1][:, sl(s)],
                                 outT[1][:, sl(s)])
            nc.vector.tensor_add(outT[1][:, sl(s)], outT[1][:, sl(s)],
                                 cT[1][:, sl(s)])
            if s == 2:
                nc.sync.dma_start(out_d[1, :, 0:1536], outT[1][:, 0:1536])
            if s == 3:
                nc.sync.dma_start(out_d[1, :, 1536:2048], outT[1][:, 1536:2048])

    nc.compile()
    _CACHE["nc"] = nc
    return nc


def _prep_host(inputs, hx, adp, support_rows, support_cols, support_vals,
               W_ru, W_c):
    xcat = np.concatenate(
        [inputs.reshape(B, N, D_IN), hx.reshape(B, N, UNITS)], axis=2)
    xcat = np.ascontiguousarray(xcat, dtype=np.float32)

    S = np.zeros((N, N), np.float32)
    np.add.at(S, (support_rows, support_cols), support_vals)
    # s2[kp, p, j, n] = S[n, 128*(2kp+j)+p] * 256
    s2 = np.ascontiguousarray(
        (S.T * S_SCALE).reshape(KP, 2, 128, N).transpose(0, 2, 1, 3)
    ).astype(FP8)

    # adp2[b, kp, p, j, n] = adp[b, n, 128*(2kp+j)+p] * 32768
    adp2 = np.ascontiguousarray(
        (adp.transpose(0, 2, 1) * A_SCALE).reshape(B, KP, 2, 128, N)
        .transpose(0, 1, 3, 2, 4)
    ).astype(FP8)

    # xnm[p, b, k, f] = x[b, 128k+p, f], feature-padded to FPAD
    xnm = np.zeros((B, 128, KC, FPAD), FP8)
    xnm[:, :, :, 0:F] = xcat.reshape(B, KC, 128, F).transpose(0, 2, 1, 3)
    xT = xcat.transpose(0, 2, 1)  # (B, F, N) feature-major
    hxih = np.concatenate([xT[:, D_IN:F, :], xT[:, 0:D_IN, :]], axis=1)
    hxih = np.ascontiguousarray(hxih).astype(BF16)

    wru = np.ascontiguousarray(W_ru.reshape(F, 3, 2 * UNITS)).astype(np.float32)
    wru[:, 1, :] /= S_SCALE
    wru[:, 2, :] /= A_SCALE
    wc = W_c.reshape(F, 3, UNITS).astype(np.float32)
    wc0L = wc[:, 0, :] * LAM
    wcy = np.stack(
        [wc[:, 1, :] * (LAM / S_SCALE), wc[:, 2, :] * (LAM / A_SCALE)],
        axis=1)  # [F, 2, UNITS]

    # hxi row order: [hx features (2:66) | inp features (0:2)]
    perm = np.concatenate([np.arange(D_IN, F), np.arange(0, D_IN)])
    wblob = np.zeros((128, 704), np.float32)
    wblob[0:F, 0:128] = wru[perm, 0, :]      # x0 term, hxi-permuted rows
    wblob[0:F, 128:256] = wru[:, 1, :]       # x1 term, feature-order rows
    wblob[0:F, 256:384] = wru[:, 2, :]       # x2 term, feature-order rows
    wblob[0:F, 384:512] = wcy[perm].reshape(F, 128)
    wblob[0:F, 512:576] = wc0L[perm]
    wblob[:, 576:704] = np.eye(128)

    shared = {"sT": s2, "wblob": wblob.astype(BF16)}
    in_maps = []
    for c in range(N_CORES):
        lo, hi = c * B_LOC, (c + 1) * B_LOC
        in_maps.append({
            "adpT": np.ascontiguousarray(adp2[lo:hi]),
            "xnm": np.ascontiguousarray(xnm[lo:hi].transpose(1, 0, 2, 3)),
            "hxi": np.ascontiguousarray(hxih[lo:hi]),
            **shared,
        })
    return in_maps


def kernel(inputs, hx, adp, support_rows, support_cols, support_vals,
           W_ru, W_c, time_axis=None):
    from concourse.bass_utils import run_bass_kernel_spmd

    inputs = np.asarray(inputs, dtype=np.float32)
    hx = np.asarray(hx, dtype=np.float32)
    adp = np.asarray(adp, dtype=np.float32)
    support_rows = np.asarray(support_rows)
    support_cols = np.asarray(support_cols)
    support_vals = np.asarray(support_vals, dtype=np.float32)
    W_ru = np.asarray(W_ru, dtype=np.float32)
    W_c = np.asarray(W_c, dtype=np.float32)

    nc = _build()
    in_maps = _prep_host(inputs, hx, adp, support_rows, support_cols,
                         support_vals, W_ru, W_c)

    res = run_bass_kernel_spmd(nc, in_maps, core_ids=list(range(N_CORES)),
                               trace=False)
    _CACHE["last_result"] = res

    out = np.empty((B, N * UNITS), np.float32)
    for c in range(N_CORES):
        outT = np.asarray(res.results[c]["outT"], dtype=np.float32)
        for i in range(B_LOC):
            out[c * B_LOC + i] = np.ascontiguousarray(
                outT[i].T).reshape(N * UNITS)
    return out


# revision 15
# speedup vs baseline: 1.0154x; 1.0154x over previous
"""Trainium2 Bass kernel for nn_DMGAGRUcell (GRU cell with graph-conv gates).

Math (per batch b):
  x    = [inputs | hx]                      (N, 66)
  ru   = sigmoid(x W0 + (S x) W1 + (adp x) W2);  r, u = split(ru)
  x'   = [inputs | r*hx]
  c    = tanh(x' Wc0 + (S x') Wc1 + (adp x') Wc2)
  out  = u*hx + (1-u)*c

Sharding: 2 batches per core x 8 cores (data parallel over B=16).

Device strategy:
  - All four N x N streaming products run as fp8e4 DoubleRow matmuls
    (0.5 cycles/row, 2 k-chunks per instruction).  S is scaled x256 and
    adp x32768 so fp8 values stay out of the subnormal range; the scales
    are folded into the gate weights on the host.
  - gconv1 is classic: streams produce x1T/x2T (bf16, feature-major),
    small matmuls per 512-slab accumulate the ru pre-activation.  For
    the late batch the x0/x1 terms are pre-accumulated into SBUF early
    (rupT) and re-injected with an identity matmul, so only two matmuls
    per slab remain after its adp pass lands.
  - gconv2 is weight-folded: y1 = x'(Wc1*LAM/256), y2 = x'(Wc2*LAM/32768)
    are computed node-major (tiny matmuls, inp-part + rh-part) and
    quantized to fp8; the S/adp streams then accumulate LAM*c_pre
    directly in PSUM (with the Wc0*LAM direct term), and tanh applies
    scale 1/LAM.  No second-gconv drains, no transposes.
  - hx lives at partitions 0:64 and the 2 input rows at 64:66 of one
    packed tile (HW requires 32-aligned partition bases); weight rows
    are permuted on the host to match, so every matmul/elementwise op
    has legal base pairs.  PSUM: two 4-slot rings whose allocation order
    matches the pass windows.
"""

import numpy as np
import ml_dtypes

BF16 = ml_dtypes.bfloat16
FP8 = ml_dtypes.float8_e4m3fn

N = 2048
B = 16
D_IN = 2
UNITS = 64
F = 66
B_LOC = 2          # batches per core
N_CORES = 8
KC = 16            # k chunks of 128 nodes
KP = 8             # k-chunk pairs (DoubleRow)
NS = 4             # 512-wide output slabs
FPAD = 80          # padded feature count (DoubleRow needs step % 16 == 0)

S_SCALE = 256.0    # fp8 scale for the sparse support matrix
A_SCALE = 32768.0  # fp8 scale for the adaptive adjacency (2048 * 16)
LAM = 32768.0      # common fixed-point scale of the gconv2 PSUM accumulation

_CACHE = {}


def _build():
    if "nc" in _CACHE:
        return _CACHE["nc"]

    from contextlib import ExitStack
    import concourse.mybir as mybir
    import concourse.tile as tile
    from concourse import bacc

    f32 = mybir.dt.float32
    bf = mybir.dt.bfloat16
    f8 = mybir.dt.float8e4
    AF = mybir.ActivationFunctionType
    DR = mybir.MatmulPerfMode.DoubleRow

    nc = bacc.Bacc("TRN2", target_bir_lowering=False, debug=False,
                   num_devices=N_CORES)

    adp_d = nc.dram_tensor("adpT", [B_LOC, KP, 128, 2, N], f8, kind="ExternalInput")
    s_d = nc.dram_tensor("sT", [KP, 128, 2, N], f8, kind="ExternalInput")
    xnm_d = nc.dram_tensor("xnm", [128, B_LOC, KC, FPAD], f8, kind="ExternalInput")
    # hxi: rows 0:64 = hx (feature-major), rows 64:66 = inputs
    hxi_d = nc.dram_tensor("hxi", [B_LOC, F, N], bf, kind="ExternalInput")
    # wblob cols: [0:384] wru (3x128, rows hxi-permuted for m=0),
    # [384:512] wcy (2x64), [512:576] wc0, [576:704] identity(128).
    # wcy/wc0 rows: 0:64 hx-part, 64:66 inp-part (hxi layout).
    wb_d = nc.dram_tensor("wblob", [128, 704], bf, kind="ExternalInput")
    out_d = nc.dram_tensor("outT", [B_LOC, UNITS, N], bf, kind="ExternalOutput")

    with tile.TileContext(nc) as tc, ExitStack() as ctx:
        cpool = ctx.enter_context(tc.tile_pool(name="cpool", bufs=1))
        spool = ctx.enter_context(tc.tile_pool(name="spool", bufs=1))
        apool = ctx.enter_context(tc.tile_pool(name="apool", bufs=1))
        wk = ctx.enter_context(tc.tile_pool(name="wk", bufs=1))
        pp = ctx.enter_context(tc.tile_pool(name="pp", bufs=4, space="PSUM"))

        def sl(s):
            return slice(s * 512, (s + 1) * 512)

        # ---- tiles (SBUF residency) ----
        xnm = wk.tile([128, B_LOC, KC, FPAD], f8, tag="xnm", name="xnm")
        hxi = [wk.tile([F, N], bf, tag=f"hxi{b}", name=f"hxi{b}")
               for b in range(B_LOC)]
        s2 = [spool.tile([128, 2, N], f8, tag=f"s{kp}", name=f"s{kp}")
              for kp in range(KP)]
        a2 = [[apool.tile([128, 2, N], f8, tag=f"a{b}_{kp}", name=f"a{b}_{kp}")
               for kp in range(KP)] for b in range(B_LOC)]
        wb = cpool.tile([128, 704], bf, tag="wb", name="wb")

        def wru_hx0():
            return wb[0:UNITS, 0:128]

        def wru_inp0():
            return wb[UNITS:F, 0:128]

        def wru(m):
            return wb[0:F, 128 * m:128 * (m + 1)]

        def wcyR(j):
            return wb[0:UNITS, 384 + 64 * j:384 + 64 * (j + 1)]

        def wcyI(j):
            return wb[UNITS:F, 384 + 64 * j:384 + 64 * (j + 1)]

        wc0R = wb[0:UNITS, 512:576]
        wc0I = wb[UNITS:F, 512:576]
        ident = wb[:, 576:704]

        x1T = [wk.tile([F, N], bf, tag=f"x1T{b}", name=f"x1T{b}") for b in range(B_LOC)]
        x2T = [wk.tile([F, N], bf, tag=f"x2T{b}", name=f"x2T{b}") for b in range(B_LOC)]
        rupT = wk.tile([128, N], bf, tag="rupT", name="rupT")
        rhT = [wk.tile([UNITS, N], bf, tag=f"rhT{b}", name=f"rhT{b}") for b in range(B_LOC)]
        ract = [wk.tile([UNITS, N], bf, tag=f"ract{b}", name=f"ract{b}") for b in range(B_LOC)]
        uact = [wk.tile([UNITS, N], bf, tag=f"uact{b}", name=f"uact{b}") for b in range(B_LOC)]
        cT = [wk.tile([UNITS, N], bf, tag=f"cT{b}", name=f"cT{b}") for b in range(B_LOC)]
        outT = [wk.tile([UNITS, N], bf, tag=f"outT{b}", name=f"outT{b}") for b in range(B_LOC)]
        y1nm = [wk.tile([128, KC, UNITS], f8, tag=f"y1nm{b}", name=f"y1nm{b}")
                for b in range(B_LOC)]
        y2nm = [wk.tile([128, KC, UNITS], f8, tag=f"y2nm{b}", name=f"y2nm{b}")
                for b in range(B_LOC)]

        # ---- DMA issue order == HBM arrival order (one serialized bus,
        # ~0.65us HWDGE serialization per DMA, so smalls are packed).
        nc.sync.dma_start(wb[:], wb_d[:])
        nc.sync.dma_start(xnm[:], xnm_d[:])
        for kp in range(KP):
            nc.sync.dma_start(s2[kp][:], s_d[kp])
        nc.sync.dma_start(hxi[0][:], hxi_d[0])
        nc.sync.dma_start(hxi[1][:], hxi_d[1])
        for kp in range(KP):
            nc.sync.dma_start(a2[0][kp][:], adp_d[0, kp])
        for kp in range(KP):
            nc.sync.dma_start(a2[1][kp][:], adp_d[1, kp])

        # Warm the ACT sigmoid/tanh function table off the critical path
        # (the first Sigmoid otherwise pays a ~1.3us table load mid-kernel).
        dum = cpool.tile([1, 2], f32, tag="dum", name="dum")
        nc.scalar.activation(dum[0:1, 0:1], wb[0:1, 0:1], AF.Sigmoid)

        def drain(dst, src, s):
            # PSUM -> SBUF copies: only DVE and ACT may read PSUM.
            if s % 2 == 0:
                nc.vector.tensor_copy(dst, src)
            else:
                nc.scalar.activation(dst, src, AF.Copy)

        def dr_mm(ps, lhsT_tile, rhs_tile, s, start, stop):
            nc.tensor.matmul(ps[:], lhsT_tile, rhs_tile[:, :, sl(s)],
                             start=start, stop=stop, perf_mode=DR)

        def xnm_pair(b, kp):
            return xnm[:, b, 2 * kp:2 * kp + 2, :]

        def ru_open_mms(ps, b, s, stop_last):
            # x0 term (split into hx + inp parts) and the x1 term.
            nc.tensor.matmul(ps[:], wru_hx0(), hxi[b][0:UNITS, sl(s)],
                             start=True, stop=False)
            nc.tensor.matmul(ps[:], wru_inp0(), hxi[b][UNITS:F, sl(s)],
                             start=False, stop=False)
            nc.tensor.matmul(ps[:], wru(1), x1T[b][:, sl(s)],
                             start=False, stop=stop_last)

        def ymm(b, psY, chunks, which):
            # y{1,2} = x' @ Wcy_{1,2}, node-major: per 128-node chunk,
            # two tiny matmuls (inp rows + rh rows) accumulate [128, 64].
            for k in chunks:
                ck = slice(128 * k, 128 * (k + 1))
                dst = psY[k // 8][:, k % 8, :]
                nc.tensor.matmul(dst, hxi[b][UNITS:F, ck], wcyI(which),
                                 start=True, stop=False)
                nc.tensor.matmul(dst, rhT[b][:, ck], wcyR(which),
                                 start=False, stop=True)

        def ynm_copies(b, psY1, psY2, y2_on_act=False):
            # quantize y1/y2 to fp8 node-major, 4-chunk granularity; all of
            # y1 first (the S stream consumes it first).
            for yd, ps, act in ((y1nm[b], psY1, False), (y2nm[b], psY2, y2_on_act)):
                for q in range(4):
                    src = ps[q // 2][:, 4 * (q % 2):4 * (q % 2) + 4, :]
                    dst = yd[:, 4 * q:4 * q + 4, :]
                    if act:
                        nc.scalar.activation(dst, src, AF.Copy)
                    else:
                        nc.vector.tensor_copy(dst, src)

        def cterm0(psC, b, s):
            nc.tensor.matmul(psC[s][:], wc0I, hxi[b][UNITS:F, sl(s)],
                             start=True, stop=False)
            nc.tensor.matmul(psC[s][:], wc0R, rhT[b][:, sl(s)],
                             start=False, stop=False)

        def final_ops(b, s):
            nc.vector.tensor_sub(outT[b][:, sl(s)], hxi[b][0:UNITS, sl(s)],
                                 cT[b][:, sl(s)])
            nc.vector.tensor_mul(outT[b][:, sl(s)], uact[b][:, sl(s)],
                                 outT[b][:, sl(s)])
            nc.vector.tensor_add(outT[b][:, sl(s)], outT[b][:, sl(s)],
                                 cT[b][:, sl(s)])

        # ============ gconv1 S passes, both batches pair-paced ============
        psS0 = [pp.tile([FPAD, 512], f32, tag="pA", name=f"psS0_{s}")
                for s in range(NS)]
        psS1 = [pp.tile([FPAD, 512], f32, tag="pB", name=f"psS1_{s}")
                for s in range(NS)]
        for kp in range(KP):
            for s in range(NS):
                dr_mm(psS0[s], xnm_pair(0, kp), s2[kp], s, kp == 0, kp == KP - 1)
            for s in range(NS):
                dr_mm(psS1[s], xnm_pair(1, kp), s2[kp], s, kp == 0, kp == KP - 1)
        for s in range(NS):
            drain(x1T[0][:, sl(s)], psS0[s][0:F, :], s)
        for s in range(NS):
            drain(x1T[1][:, sl(s)], psS1[s][0:F, :], s + 1)

        # b1 ru partial: x0 + x1 terms accumulated early, drained to SBUF.
        rup1 = [pp.tile([128, 512], f32, tag="pB", name=f"rup1_{s}")
                for s in range(NS)]
        for s in range(NS):
            ru_open_mms(rup1[s], 1, s, stop_last=True)
        for s in range(NS):
            drain(rupT[:, sl(s)], rup1[s][:], s)

        # ru b0 opens with the x0/x1 terms (ring B, freed by sigmoids).
        ru0 = [pp.tile([128, 512], f32, tag="pB", name=f"ru0_{s}")
               for s in range(NS)]
        for s in range(NS):
            ru_open_mms(ru0[s], 0, s, stop_last=False)

        # ============ gconv1 adp pass b0 (pair-paced, ring A) ============
        psA0 = [pp.tile([FPAD, 512], f32, tag="pA", name=f"psA0_{s}")
                for s in range(NS)]
        for kp in range(KP):
            for s in range(NS):
                dr_mm(psA0[s], xnm_pair(0, kp), a2[0][kp], s, kp == 0, kp == KP - 1)

        # Per-slab: drain x2T -> close ru -> sigmoid(r) -> rh, pipelined.
        for s in range(NS):
            drain(x2T[0][:, sl(s)], psA0[s][0:F, :], s)
            nc.tensor.matmul(ru0[s][:], wru(2), x2T[0][:, sl(s)],
                             start=False, stop=True)
            nc.scalar.activation(ract[0][:, sl(s)], ru0[s][0:UNITS, :],
                                 AF.Sigmoid)
            nc.vector.tensor_mul(rhT[0][:, sl(s)], ract[0][:, sl(s)],
                                 hxi[0][0:UNITS, sl(s)])
        # u sigmoids as a block (off the rh critical path; they release the
        # ru0 slots that psC0 reuses).
        for s in range(NS):
            nc.scalar.activation(uact[0][:, sl(s)], ru0[s][UNITS:128, :],
                                 AF.Sigmoid)

        # y1/y2 for b0 (ring A after psA0's drains release its slots).
        psY0 = [pp.tile([128, 8, UNITS], f32, tag="pA", name=f"psY0_{i}")
                for i in range(4)]
        ymm(0, psY0[0:2], range(KC), 0)
        ymm(0, psY0[2:4], range(KC), 1)
        ynm_copies(0, psY0[0:2], psY0[2:4])

        # b1 gconv1 adp pass (ring A after psY0's ynm copies release slots;
        # the early pairs are resident by then, the rest are DMA-paced).
        psA1 = [pp.tile([FPAD, 512], f32, tag="pA", name=f"psA1_{s}")
                for s in range(NS)]

        def a1_block(kp):
            for s in range(NS):
                dr_mm(psA1[s], xnm_pair(1, kp), a2[1][kp], s,
                      kp == 0, kp == KP - 1)

        a1_block(0)
        a1_block(1)

        # ====== b0 gconv2 (ring B): kp-major S-section consumes ynm as it
        # lands; slab-major adp section pipelines tanh/finals; b1's adp
        # pairs and ru close are woven in by DMA arrival time. ======
        psC0 = [pp.tile([UNITS, 512], f32, tag="pB", name=f"psC0_{s}")
                for s in range(NS)]
        for s in range(NS):
            cterm0(psC0, 0, s)
        for kp in range(KP):
            for s in range(NS):
                dr_mm(psC0[s], y1nm[0][:, 2 * kp:2 * kp + 2, :], s2[kp], s,
                      False, False)
            if kp == 1:
                a1_block(2)
            if kp == 4:
                a1_block(3)
            if kp == 7:
                a1_block(4)
        ru1 = [pp.tile([128, 512], f32, tag="pA", name=f"ru1_{q}")
               for q in range(NS)]
        for s in range(NS):
            for kp in range(KP):
                dr_mm(psC0[s], y2nm[0][:, 2 * kp:2 * kp + 2, :], a2[0][kp], s,
                      False, kp == KP - 1)
            nc.scalar.activation(cT[0][:, sl(s)], psC0[s][:],
                                 AF.Tanh, scale=1.0 / LAM)
            nc.gpsimd.tensor_sub(outT[0][:, sl(s)], hxi[0][0:UNITS, sl(s)],
                                 cT[0][:, sl(s)])
            nc.gpsimd.tensor_mul(outT[0][:, sl(s)], uact[0][:, sl(s)],
                                 outT[0][:, sl(s)])
            nc.gpsimd.tensor_add(outT[0][:, sl(s)], outT[0][:, sl(s)],
                                 cT[0][:, sl(s)])
            if s == 0:
                a1_block(5)
            if s == 1:
                a1_block(6)
                nc.sync.dma_start(out_d[0, :, 0:1024], outT[0][:, 0:1024])
            if s == 2:
                a1_block(7)
                # b1 gconv1 close: drains all-DVE, ident re-injects the
                # early partial, then the x2 term; r/u sigmoids follow.
                for q in range(NS):
                    drain(x2T[1][:, sl(q)], psA1[q][0:F, :], q)
                    nc.tensor.matmul(ru1[q][:], ident, rupT[:, sl(q)],
                                     start=True, stop=False)
                    nc.tensor.matmul(ru1[q][:], wru(2), x2T[1][:, sl(q)],
                                     start=False, stop=True)
                    nc.scalar.activation(ract[1][:, sl(q)], ru1[q][0:UNITS, :],
                                         AF.Sigmoid)
                    nc.vector.tensor_mul(rhT[1][:, sl(q)], ract[1][:, sl(q)],
                                         hxi[1][0:UNITS, sl(q)])
                    nc.scalar.activation(uact[1][:, sl(q)],
                                         ru1[q][UNITS:128, :],
                                         AF.Sigmoid, scale=-1.0)
            if s == 3:
                nc.sync.dma_start(out_d[0, :, 1024:2048], outT[0][:, 1024:2048])

        # ---- b1 y/ynm (psY1 on ring A after ru1's sigmoids) ----
        psY1 = [pp.tile([128, 8, UNITS], f32, tag="pA", name=f"psY1_{i}")
                for i in range(4)]
        ymm(1, psY1[0:2], range(KC), 0)
        ymm(1, psY1[2:4], range(KC), 1)
        for q in range(4):
            drain(y1nm[1][:, 4 * q:4 * q + 4, :],
                  psY1[q // 2][:, 4 * (q % 2):4 * (q % 2) + 4, :], q)
        for q in range(4):
            drain(y2nm[1][:, 4 * q:4 * q + 4, :],
                  psY1[2 + q // 2][:, 4 * (q % 2):4 * (q % 2) + 4, :], q + 1)

        # p = u*hx = hx - u'*hx precomputed into ract1 (dead after rh), so
        # only two elementwise ops remain after each b1 tanh.
        for s in range(NS):
            nc.vector.tensor_mul(ract[1][:, sl(s)], uact[1][:, sl(s)],
                                 hxi[1][0:UNITS, sl(s)])
            nc.vector.tensor_sub(ract[1][:, sl(s)], hxi[1][0:UNITS, sl(s)],
                                 ract[1][:, sl(s)])

        # ---- b1 gconv2 (ring B): kp-major S, slab-major adp + tail ----
        psC1 = [pp.tile([UNITS, 512], f32, tag="pB", name=f"psC1_{s}")
                for s in range(NS)]
        for s in range(NS):
            cterm0(psC1, 1, s)
        for kp in range(KP):
            for s in range(NS):
                dr_mm(psC1[s], y1nm[1][:, 2 * kp:2 * kp + 2, :], s2[kp], s,
                      False, False)
        for s in range(NS):
            for kp in range(KP):
                dr_mm(psC1[s], y2nm[1][:, 2 * kp:2 * kp + 2, :], a2[1][kp], s,
                      False, kp == KP - 1)
            nc.scalar.activation(cT[1][:, sl(s)], psC1[s][:],
                                 AF.Tanh, scale=1.0 / LAM)
            nc.vector.tensor_mul(outT[1][:, sl(s)], uact[1][:, sl(s)],
                                 cT[1][:, sl(s)])
            nc.vector.tensor_add(outT[1][:, sl(s)], outT[1][:, sl(s)],
                                 ract[1][:, sl(s)])
            if s == 2:
                nc.sync.dma_start(out_d[1, :, 0:1536], outT[1][:, 0:1536])
            if s == 3:
                nc.sync.dma_start(out_d[1, :, 1536:2048], outT[1][:, 1536:2048])

    nc.compile()
    _CACHE["nc"] = nc
    return nc


def _prep_host(inputs, hx, adp, support_rows, support_cols, support_vals,
               W_ru, W_c):
    xcat = np.concatenate(
        [inputs.reshape(B, N, D_IN), hx.reshape(B, N, UNITS)], axis=2)
    xcat = np.ascontiguousarray(xcat, dtype=np.float32)

    S = np.zeros((N, N), np.float32)
    np.add.at(S, (support_rows, support_cols), support_vals)
    # s2[kp, p, j, n] = S[n, 128*(2kp+j)+p] * 256
    s2 = np.ascontiguousarray(
        (S.T * S_SCALE).reshape(KP, 2, 128, N).transpose(0, 2, 1, 3)
    ).astype(FP8)

    # adp2[b, kp, p, j, n] = adp[b, n, 128*(2kp+j)+p] * 32768
    adp2 = np.ascontiguousarray(
        (adp.transpose(0, 2, 1) * A_SCALE).reshape(B, KP, 2, 128, N)
        .transpose(0, 1, 3, 2, 4)
    ).astype(FP8)

    # xnm[p, b, k, f] = x[b, 128k+p, f], feature-padded to FPAD
    xnm = np.zeros((B, 128, KC, FPAD), FP8)
    xnm[:, :, :, 0:F] = xcat.reshape(B, KC, 128, F).transpose(0, 2, 1, 3)
    xT = xcat.transpose(0, 2, 1)  # (B, F, N) feature-major
    hxih = np.concatenate([xT[:, D_IN:F, :], xT[:, 0:D_IN, :]], axis=1)
    hxih = np.ascontiguousarray(hxih).astype(BF16)

    wru = np.ascontiguousarray(W_ru.reshape(F, 3, 2 * UNITS)).astype(np.float32)
    wru[:, 1, :] /= S_SCALE
    wru[:, 2, :] /= A_SCALE
    wc = W_c.reshape(F, 3, UNITS).astype(np.float32)
    wc0L = wc[:, 0, :] * LAM
    wcy = np.stack(
        [wc[:, 1, :] * (LAM / S_SCALE), wc[:, 2, :] * (LAM / A_SCALE)],
        axis=1)  # [F, 2, UNITS]

    # hxi row order: [hx features (2:66) | inp features (0:2)]
    perm = np.concatenate([np.arange(D_IN, F), np.arange(0, D_IN)])
    wblob = np.zeros((128, 704), np.float32)
    wblob[0:F, 0:128] = wru[perm, 0, :]      # x0 term, hxi-permuted rows
    wblob[0:F, 128:256] = wru[:, 1, :]       # x1 term, feature-order rows
    wblob[0:F, 256:384] = wru[:, 2, :]       # x2 term, feature-order rows
    wblob[0:F, 384:512] = wcy[perm].reshape(F, 128)
    wblob[0:F, 512:576] = wc0L[perm]
    wblob[:, 576:704] = np.eye(128)

    shared = {"sT": s2, "wblob": wblob.astype(BF16)}
    in_maps = []
    for c in range(N_CORES):
        lo, hi = c * B_LOC, (c + 1) * B_LOC
        in_maps.append({
            "adpT": np.ascontiguousarray(adp2[lo:hi]),
            "xnm": np.ascontiguousarray(xnm[lo:hi].transpose(1, 0, 2, 3)),
            "hxi": np.ascontiguousarray(hxih[lo:hi]),
            **shared,
        })
    return in_maps


def kernel(inputs, hx, adp, support_rows, support_cols, support_vals,
           W_ru, W_c, time_axis=None):
    from concourse.bass_utils import run_bass_kernel_spmd

    inputs = np.asarray(inputs, dtype=np.float32)
    hx = np.asarray(hx, dtype=np.float32)
    adp = np.asarray(adp, dtype=np.float32)
    support_rows = np.asarray(support_rows)
    support_cols = np.asarray(support_cols)
    support_vals = np.asarray(support_vals, dtype=np.float32)
    W_ru = np.asarray(W_ru, dtype=np.float32)
    W_c = np.asarray(W_c, dtype=np.float32)

    nc = _build()
    in_maps = _prep_host(inputs, hx, adp, support_rows, support_cols,
                         support_vals, W_ru, W_c)

    res = run_bass_kernel_spmd(nc, in_maps, core_ids=list(range(N_CORES)),
                               trace=False)
    _CACHE["last_result"] = res

    out = np.empty((B, N * UNITS), np.float32)
    for c in range(N_CORES):
        outT = np.asarray(res.results[c]["outT"], dtype=np.float32)
        for i in range(B_LOC):
            out[c * B_LOC + i] = np.ascontiguousarray(
                outT[i].T).reshape(N * UNITS)
    return out
